# revision 12
# baseline (speedup 1.0000x reference)
"""LIF spiking layer (T=32, B=256, C_in=C_out=4096, fp32) on 8 trn2 NeuronCores.

Strategy: data-parallel over batch (32 samples/core, W replicated). Host-side
numpy pre-permutes operands into SBUF tile layout (contraction dim ci on
partitions); each core runs matmuls + the LIF recurrence:

  current[co, (t,b)] = W @ x_core.T  per 128-co tile, accumulated in psum from
    1) an fp16 hi pass        Wh @ Xh          (32 matmuls of 512 cols), and
    2) ONE fp8 DoubleRow pass rw8@Xh8 + W8@rx8 (the two correction terms ride
       the DoubleRow pair dim at 0.5 cycles/row — half the cost of a pass),
  so the matmul costs 1.5 fp16-pass-equivalents instead of fp16x3's 3.0.

  The identity: with Xh = fp16(x*2^8), rx = x*2^8 - Xh, Wh = fp16(W*2^9),
  rw = W*2^9 - Wh:  2^17*(W@x) = Wh@Xh + rw@Xh + (W*2^9)@rx  exactly; the two
  correction operand pairs are stored in fp8-e4m3 with power-of-2 scales that
  cancel per DoubleRow slot, so all passes accumulate into one psum at 2^17.
  (Measured on the real inputs: 177/33.5M spike flips, rel err ~6e-3.)

  Overlap structure: the first THREE co-tiles run hi then corr k-strip-
  dribbled (k outer, co-tile inner) so the PE has ~61us of issueable work
  while the 16.8MB x stream lands — hides the input DMA. Per group of two
  co-tiles the Activation engine copies psum->SBUF, and VectorE runs the LIF
  recurrence on SBUF with one fused [128,2,32] op per step; psum banks
  (4 per group, 8 total) recycle as soon as the ACT copy drains them.

Spikes leave as fp16 in [co, (t,b)] layout; the host transposes + casts.
"""

import os

import numpy as np
import ml_dtypes

import concourse.mybir as mybir
import concourse.tile as tile
from concourse import bacc
from concourse.bass_utils import run_bass_kernel_spmd

FP32 = mybir.dt.float32
FP16 = mybir.dt.float16
FP8 = mybir.dt.float8e4
E4M3 = ml_dtypes.float8_e4m3

N_CORES = 8
T, B, CI, CO = 32, 256, 4096, 4096
B_LOC = B // N_CORES  # 32
TB = T * B_LOC  # 1024
# Power-of-2 scales: the LIF recurrence is exactly scale-equivariant. The
# hi pass and both fp8 correction slots all land at product scale 2^17.
SX = 2.0**8
SW = 2.0**9
SCALE = SX * SW  # 2^17

TRACE = False
LAST_EXEC_NS = None
MODE = os.environ.get("LIF_KERNEL_MODE", "fp16fp8")

_CACHE = {}


def build_kernel_fp16fp8(
    d: float,
    th: float,
    has_bias: bool,
    T=T,
    B_loc=B_LOC,
    CI=CI,
    CO=CO,
):
    """fp16 hi pass + single fp8-e4m3 DoubleRow correction pass (1.5x fp16
    pass cost); ACT stages psum->SBUF; fused recurrence on VectorE."""
    TBl = T * B_loc
    n_k = CI // 128
    n_c = CO // 128
    csize = 512
    n_chunk = TBl // csize
    ths = float(th) * SCALE
    PRO = 3  # co-tiles in the k-dribbled prologue

    nc = bacc.Bacc("TRN2", target_bir_lowering=False, debug=False, num_devices=N_CORES)

    xh = nc.declare_dram_parameter("xh", [128, n_k, TBl], FP16, isOutput=False)
    xc8 = nc.declare_dram_parameter("xc8", [128, n_k, 2, TBl], FP8, isOutput=False)
    wh = nc.declare_dram_parameter("wh", [n_c, 128, n_k, 128], FP16, isOutput=False)
    wc8 = nc.declare_dram_parameter(
        "wc8", [n_c, 128, n_k, 2, 128], FP8, isOutput=False
    )
    if has_bias:
        bias = nc.declare_dram_parameter("bias", [CO, 1], FP32, isOutput=False)
    spkT = nc.declare_dram_parameter("spkT", [CO, TBl], FP16, isOutput=True)

    with tile.TileContext(nc) as tc:
        with (
            tc.tile_pool(name="xt", bufs=1) as xt_pool,
            tc.tile_pool(name="wt", bufs=PRO) as wt_pool,
            tc.tile_pool(name="st", bufs=2) as st_pool,
            tc.tile_pool(name="cs", bufs=2) as cs_pool,
            tc.tile_pool(name="mm", bufs=1) as mm_pool,
            tc.tile_pool(name="pc", bufs=8, space="PSUM") as pc_pool,
        ):
            XH = xt_pool.tile([128, n_k, TBl], FP16)
            XC = xt_pool.tile([128, n_k, 2, TBl], FP8)
            # W strips on the SP hwdge queue; X bulk on the Activation hwdge
            # queue, so W never queues behind the 16.8MB X stream.
            WH_first = wt_pool.tile([128, n_k, 128], FP16, tag="wh")
            WC_first = wt_pool.tile([128, n_k, 2, 128], FP8, tag="wc")
            wq = 8
            for kq in range(0, n_k, wq):
                nc.sync.dma_start(
                    out=WH_first[:, kq : kq + wq, :], in_=wh[0, :, kq : kq + wq, :]
                )
            nc.sync.dma_start(out=WC_first, in_=wc8[0, :, :, :, :])
            # per-k strips so the prologue consumes tiles in DMA arrival order
            for k in range(n_k):
                nc.scalar.dma_start(out=XH[:, k, :], in_=xh[:, k, :])
            for k in range(n_k):
                nc.scalar.dma_start(out=XC[:, k, :, :], in_=xc8[:, k, :, :])

            mem = mm_pool.tile([128, n_c, B_loc], FP32)
            nc.vector.memset(mem, 0.0)

            def getW(c):
                if c == 0:
                    return WH_first, WC_first
                WH_c = wt_pool.tile([128, n_k, 128], FP16, tag="wh")
                WC_c = wt_pool.tile([128, n_k, 2, 128], FP8, tag="wc")
                nc.sync.dma_start(out=WH_c, in_=wh[c, :, :, :])
                nc.sync.dma_start(out=WC_c, in_=wc8[c, :, :, :, :])
                return WH_c, WC_c

            def getB(c):
                b_tile = wt_pool.tile([128, 1], FP32, tag="bt")
                nc.sync.dma_start(out=b_tile, in_=bias[c * 128 : (c + 1) * 128, :])
                return b_tile

            def new_pcs():
                return [
                    pc_pool.tile([128, csize], FP32, tag="pc", name="pc")
                    for _ in range(n_chunk)
                ]

            def hi(pc_t, Wt, k, ch, start):
                nc.tensor.matmul(
                    pc_t,
                    lhsT=Wt[:, k, :],
                    rhs=XH[:, k, ch * csize : (ch + 1) * csize],
                    start=start,
                    stop=False,
                )

            def corr(pc_t, Wt, k, ch, stop):
                nc.tensor.matmul(
                    pc_t,
                    lhsT=Wt[:, k, :, :],
                    rhs=XC[:, k, :, ch * csize : (ch + 1) * csize],
                    start=False,
                    stop=stop,
                    perf_mode=mybir.MatmulPerfMode.DoubleRow,
                )

            def rec(g, pcs, bts, lastsplit=False):
                """LIF recurrence for co-tiles (2g, 2g+1): ACT copies psum to
                SBUF (freeing the banks), DVE runs fused [128,2,32] steps.
                lastsplit (final group): per-co-tile chains on DVE + GPSIMD so
                the even tile's chain hides under the odd tile's matmuls."""
                s_g = st_pool.tile([128, 2, TBl], FP16, tag="s")
                cur_g = cs_pool.tile([128, 2, TBl], FP32, tag="cur")
                for l in range(2):
                    for ch in range(n_chunk):
                        nc.scalar.copy(
                            out=cur_g[:, l, ch * csize : (ch + 1) * csize],
                            in_=pcs[l][ch][:, :],
                        )
                if lastsplit and not has_bias:
                    for l in range(2):
                        eng = nc.vector if l == 0 else nc.gpsimd
                        for t in range(T):
                            o = t * B_loc
                            eng.scalar_tensor_tensor(
                                out=mem[:, 2 * g + l, :],
                                in0=mem[:, 2 * g + l, :],
                                scalar=d,
                                in1=cur_g[:, l, o : o + B_loc],
                                op0=mybir.AluOpType.mult,
                                op1=mybir.AluOpType.add,
                            )
                            eng.tensor_scalar(
                                s_g[:, l, o : o + B_loc],
                                mem[:, 2 * g + l, :],
                                ths,
                                None,
                                mybir.AluOpType.is_gt,
                            )
                            eng.scalar_tensor_tensor(
                                out=mem[:, 2 * g + l, :],
                                in0=s_g[:, l, o : o + B_loc],
                                scalar=-ths,
                                in1=mem[:, 2 * g + l, :],
                                op0=mybir.AluOpType.mult,
                                op1=mybir.AluOpType.add,
                            )
                    for l in range(2):
                        c = 2 * g + l
                        nc.scalar.dma_start(
                            out=spkT[c * 128 : (c + 1) * 128, :], in_=s_g[:, l, :]
                        )
                    return
                for t in range(T):
                    o = t * B_loc
                    if has_bias:
                        # per-co-tile bias: fall back to split ops
                        for l in range(2):
                            nc.vector.scalar_tensor_tensor(
                                out=mem[:, 2 * g + l, :],
                                in0=mem[:, 2 * g + l, :],
                                scalar=d,
                                in1=cur_g[:, l, o : o + B_loc],
                                op0=mybir.AluOpType.mult,
                                op1=mybir.AluOpType.add,
                            )
                            nc.vector.tensor_scalar(
                                mem[:, 2 * g + l, :],
                                mem[:, 2 * g + l, :],
                                bts[l],
                                None,
                                mybir.AluOpType.add,
                            )
                    else:
                        nc.vector.scalar_tensor_tensor(
                            out=mem[:, 2 * g : 2 * g + 2, :],
                            in0=mem[:, 2 * g : 2 * g + 2, :],
                            scalar=d,
                            in1=cur_g[:, :, o : o + B_loc],
                            op0=mybir.AluOpType.mult,
                            op1=mybir.AluOpType.add,
                        )
                    nc.vector.tensor_scalar(
                        s_g[:, :, o : o + B_loc],
                        mem[:, 2 * g : 2 * g + 2, :],
                        ths,
                        None,
                        mybir.AluOpType.is_gt,
                    )
                    nc.vector.scalar_tensor_tensor(
                        out=mem[:, 2 * g : 2 * g + 2, :],
                        in0=s_g[:, :, o : o + B_loc],
                        scalar=-ths,
                        in1=mem[:, 2 * g : 2 * g + 2, :],
                        op0=mybir.AluOpType.mult,
                        op1=mybir.AluOpType.add,
                    )
                for l in range(2):
                    c = 2 * g + l
                    nc.scalar.dma_start(
                        out=spkT[c * 128 : (c + 1) * 128, :], in_=s_g[:, l, :]
                    )

            # ---- prologue: first PRO co-tiles k-dribbled (hi then corr) ----
            Wp = [getW(c) for c in range(PRO)]
            Bp = [getB(c) for c in range(PRO)] if has_bias else [None] * PRO
            pcp = [new_pcs() for _ in range(PRO)]
            for k in range(n_k):
                for c in range(PRO):
                    for ch in range(n_chunk):
                        hi(pcp[c][ch], Wp[c][0], k, ch, k == 0)
            for k in range(n_k):
                for c in range(PRO):
                    for ch in range(n_chunk):
                        corr(pcp[c][ch], Wp[c][1], k, ch, k == n_k - 1)
            rec(0, pcp[:2], Bp[:2])
            hold = pcp[2]  # co-tile 2's psums, consumed by group 1
            hold_b = Bp[2]

            # ---- steady state: groups of two co-tiles ----
            for g in range(1, n_c // 2):
                pcs = []
                bts = []
                for l in range(2):
                    c = 2 * g + l
                    if hold is not None and c == PRO - 1:
                        pcs.append(hold)
                        bts.append(hold_b)
                        hold = None
                        continue
                    _, WC_c = W_c = getW(c)
                    bts.append(getB(c) if has_bias else None)
                    pcs.append(new_pcs())
                    for ch in range(n_chunk):
                        for k in range(n_k):
                            hi(pcs[l][ch], W_c[0], k, ch, k == 0)
                        for k in range(n_k):
                            corr(pcs[l][ch], WC_c, k, ch, k == n_k - 1)
                # (NOTE: a per-co-tile DVE/GPSIMD split of the last group's
                # chain wins ~1.1us in sim but walrus rejects TensorScalar on
                # the Pool engine — GPSIMD lacks these opcodes on trn2.)
                rec(g, pcs, bts)

    nc.compile()
    return nc


def _xt_layout(xs):
    """[TB, CI] -> [128, CI//128, TB] so SBUF partition p holds ci = k*128+p."""
    TBl, CIl = xs.shape
    return np.ascontiguousarray(
        xs.reshape(TBl, CIl // 128, 128).transpose(2, 1, 0)
    )


def _wt_layout(Wm):
    """[CO, CI] -> [CO//128, 128, CI//128, 128]: strip c, partition p=ci%128,
    k=ci//128, j=co%128 -> W[c*128+j, k*128+p]."""
    COl, CIl = Wm.shape
    return np.ascontiguousarray(
        Wm.reshape(COl // 128, 128, CIl // 128, 128).transpose(0, 3, 2, 1)
    )


def kernel(x, W, b, decay, thresh):
    global LAST_EXEC_NS
    x = np.ascontiguousarray(np.asarray(x, dtype=np.float32))
    W = np.ascontiguousarray(np.asarray(W, dtype=np.float32))
    b = np.asarray(b, dtype=np.float32)
    decay = np.asarray(decay, dtype=np.float32)
    thresh = np.asarray(thresh, dtype=np.float32)

    d = float(decay.reshape(-1)[0])
    th = float(thresh.reshape(-1)[0])
    has_bias = bool(np.any(b != 0))

    key = (MODE, d, th, has_bias)
    if key not in _CACHE:
        _CACHE[key] = build_kernel_fp16fp8(d, th, has_bias)
    nc = _CACHE[key]

    # host-side splits (W once, x per core)
    Wh16 = (W * np.float32(SW)).astype(np.float16)
    rw = W * np.float32(SW) - Wh16.astype(np.float32)
    rw8 = (rw * np.float32(2.0**9)).astype(E4M3)
    W8 = (W * np.float32(2.0**6)).astype(E4M3)
    wh_l = _wt_layout(Wh16)
    # [n_c, 128, n_k, 2, 128]: DoubleRow slot dim inside k (small AP strides)
    wc8_l = np.ascontiguousarray(
        np.stack([_wt_layout(rw8), _wt_layout(W8)], axis=3)
    )

    in_maps = []
    for i in range(N_CORES):
        xs_i = x[:, i * B_LOC : (i + 1) * B_LOC, :].reshape(TB, CI)
        Xh16 = (xs_i * np.float32(SX)).astype(np.float16)
        rx = xs_i * np.float32(SX) - Xh16.astype(np.float32)
        Xh8 = (Xh16.astype(np.float32) * np.float32(2.0**-9)).astype(E4M3)
        rx8 = (rx * np.float32(2.0**3)).astype(E4M3)
        m = {
            "xh": _xt_layout(Xh16),
            "xc8": np.ascontiguousarray(
                np.stack([_xt_layout(Xh8), _xt_layout(rx8)], axis=2)
            ),
            "wh": wh_l,
            "wc8": wc8_l,
        }
        if has_bias:
            m["bias"] = np.ascontiguousarray(
                (b * np.float32(SCALE)).reshape(CO, 1)
            )
        in_maps.append(m)

    res = run_bass_kernel_spmd(
        nc, in_maps, core_ids=list(range(N_CORES)), trace=TRACE
    )
    LAST_EXEC_NS = res.exec_time_ns

    # spikes come back [CO, TB] fp16; transpose to [T, B_loc, CO] fp32
    out = np.concatenate(
        [
            np.ascontiguousarray(r["spkT"].astype(np.float32).T).reshape(
                T, B_LOC, CO
            )
            for r in res.results
        ],
        axis=1,
    )
    return np.ascontiguousarray(out)
